# revision 1
# baseline (speedup 1.0000x reference)
"""BinaryTreeCell (binary tree LSTM cell) TRN2 Bass kernel.

Full-input contract: kernel(**inputs) takes the unsharded numpy inputs of
reference.setup_inputs() and returns (c, h), each [131072, 256] float32.

Strategy
--------
Data-parallel over the node dimension N=131072 across 8 NeuronCores
(16384 nodes/core); all weights replicated.

All 14 GEMMs (+ the reused W_fx) collapse into ONE matmul per node block:
    z   = [x, lh, rh]                 [N, 768]
    A_g = [W_g.T; Ul_g.T; Ur_g.T]     [768, 256]   per gate g in (u,i,lf,rf,o)
    pre = z @ A + b                   [N, 1280]
The per-gate 256 columns are split into two 128-column halves and packed
as 10 chunks ordered [half0: u,i,lf,rf,o | half1: u,i,lf,rf,o] so the
elementwise stage for one feature half can start as soon as its 5 gates
are done.

On-chip layout is transposed (features on partitions, nodes on the free
dim): the host feeds zT [768, 16384], lcT/rcT [256, 16384] per core and
receives cT/hT [256, 16384], so the kernel needs zero on-chip transposes
and every DMA is wide and contiguous per partition.  Matmuls run in
float32r (full-rate on the PE at free-dim 512, ~1e-4 relative error),
accumulation in fp32 PSUM over K=768 (6 chunks of 128).  Gate
activations run on ScalarE straight out of PSUM with the per-partition
bias folded in; c and h are computed on VectorE (6 tensor_tensor ops per
feature half, plus 2 pre-adds that re-inject the once-computed shared
W_fx projection into the lf/rf gates).
"""

import numpy as np

N_TOTAL = 131072
D = 256
CORES = 8
NP_ = N_TOTAL // CORES          # 16384 nodes per core
KD = 3 * D                      # 768 contraction
KC = KD // 128                  # 6 contraction chunks
GD = 5 * D                      # 1280 gate columns
BM = 512                        # node-block (matmul free dim / PSUM bank)
NBLK = NP_ // BM                # 32 blocks per core

_CACHE = {}


def _build_nc():
    """Build + compile the per-core Bass program (same NEFF for all cores)."""
    import concourse.bass as bass
    import concourse.tile as tile
    from concourse import bacc, mybir

    f32 = mybir.dt.float32
    f32r = mybir.dt.float32r
    AF = mybir.ActivationFunctionType

    nc = bacc.Bacc("TRN2", target_bir_lowering=False, debug=False)

    zT = nc.dram_tensor("zT", [KD, NP_], f32r, kind="ExternalInput").ap()
    lcT = nc.dram_tensor("lcT", [D, NP_], f32, kind="ExternalInput").ap()
    rcT = nc.dram_tensor("rcT", [D, NP_], f32, kind="ExternalInput").ap()
    A = nc.dram_tensor("A", [10, KD, 128], f32r, kind="ExternalInput").ap()
    bias = nc.dram_tensor("bias", [128, 10], f32, kind="ExternalInput").ap()
    cT = nc.dram_tensor("cT", [D, NP_], f32, kind="ExternalOutput").ap()
    hT = nc.dram_tensor("hT", [D, NP_], f32, kind="ExternalOutput").ap()

    with tile.TileContext(nc) as tc:
        with (
            tc.tile_pool(name="wpool", bufs=1) as wpool,
            tc.tile_pool(name="zpool", bufs=4) as zpool,
            tc.tile_pool(name="cpool", bufs=3) as cpool,
            tc.tile_pool(name="gates", bufs=2) as gates,
            tc.tile_pool(name="tmp", bufs=2) as tmp,
            tc.tile_pool(name="outp", bufs=3) as outp,
            tc.tile_pool(name="psum", bufs=6, space="PSUM") as psum,
        ):
            warm = wpool.tile([128, 1], f32, tag="warm")
            nc.gpsimd.memset(warm[:], 0.0)
            warm_o = wpool.tile([128, 1], f32, tag="warm_o")
            nc.scalar.activation(warm_o[:], warm[:], AF.Tanh)
            nc.scalar.activation(warm_o[:], warm[:], AF.Sigmoid)
            b_sb = wpool.tile([128, 10], f32, tag="b")
            nc.gpsimd.dma_start(out=b_sb[:], in_=bias[:])
            A_sb = []
            for n in range(10):
                a_t = wpool.tile([128, KC, 128], f32r, tag=f"A{n}")
                if n < 5:
                    asrc = A[n].rearrange("(kc p) m -> p kc m", p=128)
                    if n == 3:
                        nc.scalar.dma_start(out=a_t[:, 2:, :], in_=asrc[:, 2:, :])
                    else:
                        nc.scalar.dma_start(out=a_t[:, 0:3, :], in_=asrc[:, 0:3, :])
                        nc.scalar.dma_start(out=a_t[:, 3:, :], in_=asrc[:, 3:, :])
                A_sb.append(a_t)
            deferred_a = [False]

            def load_rest_of_A():
                if deferred_a[0]:
                    return
                deferred_a[0] = True
                for n in range(5, 10):
                    asrc = A[n].rearrange("(kc p) m -> p kc m", p=128)
                    if n == 8:
                        nc.scalar.dma_start(out=A_sb[n][:, 2:, :], in_=asrc[:, 2:, :])
                    else:
                        nc.scalar.dma_start(
                            out=A_sb[n][:, 0:3, :], in_=asrc[:, 0:3, :]
                        )
                        nc.scalar.dma_start(out=A_sb[n][:, 3:, :], in_=asrc[:, 3:, :])

            blocks = [(i * BM, BM) for i in range(NBLK - 1)]
            last = (NBLK - 1) * BM
            blocks += [(last, BM // 2), (last + BM // 2, BM // 2)]
            for blk, (m0, bm) in enumerate(blocks):
                z_sb = zpool.tile([128, KC, bm], f32r, tag="z")
                zsrc = zT[:, m0:m0 + bm].rearrange("(kc p) m -> p kc m", p=128)
                nc.sync.dma_start(out=z_sb[:, 0:3, :], in_=zsrc[:, 0:3, :])
                nc.sync.dma_start(out=z_sb[:, 3:, :], in_=zsrc[:, 3:, :])
                lc_sb = cpool.tile([128, 2, bm], f32, tag="lc")
                nc.gpsimd.dma_start(
                    out=lc_sb[:],
                    in_=lcT[:, m0:m0 + bm].rearrange("(f p) m -> p f m", p=128),
                )
                rc_sb = cpool.tile([128, 2, bm], f32, tag="rc")
                nc.gpsimd.dma_start(
                    out=rc_sb[:],
                    in_=rcT[:, m0:m0 + bm].rearrange("(f p) m -> p f m", p=128),
                )

                for f in range(2):
                    g_sb = []
                    # u, i: full K=768 accumulation
                    for g in (0, 1):
                        n = 5 * f + g
                        ps = psum.tile([128, bm], f32, tag="mm")
                        for k in range(KC):
                            nc.tensor.matmul(
                                ps[:], A_sb[n][:, k, :], z_sb[:, k, :],
                                start=(k == 0), stop=(k == KC - 1),
                            )
                        gt = gates.tile([128, bm], f32, tag=f"g{g}")
                        nc.scalar.activation(
                            gt[:], ps[:],
                            AF.Tanh if g == 0 else AF.Sigmoid,
                            bias=b_sb[:, n:n + 1],
                        )
                        g_sb.append(gt)
                        load_rest_of_A()
                    n_lf = 5 * f + 2
                    n_rf = 5 * f + 3
                    if blk == len(blocks) - 1:
                        # final block: full K=768 for lf/rf — no DVE pre-add
                        # in the kernel's exposed tail chain (rf x-chunks are
                        # W_fx duplicates, valid contraction over all 768)
                        for g, n in ((2, n_lf), (3, n_rf)):
                            ps = psum.tile([128, bm], f32, tag="mm")
                            for k in range(KC):
                                nc.tensor.matmul(
                                    ps[:], A_sb[n_lf if k < 2 else n][:, k, :],
                                    z_sb[:, k, :],
                                    start=(k == 0), stop=(k == KC - 1),
                                )
                            gt = gates.tile([128, bm], f32, tag=f"g{g}")
                            nc.scalar.activation(
                                gt[:], ps[:], AF.Sigmoid, bias=b_sb[:, n_lf:n_lf + 1],
                            )
                            g_sb.append(gt)
                    else:
                        # fx computed once (x chunks of the lf column block)
                        ps_fx = psum.tile([128, bm], f32, tag="mm")
                        for k in (0, 1):
                            nc.tensor.matmul(
                                ps_fx[:], A_sb[n_lf][:, k, :], z_sb[:, k, :],
                                start=(k == 0), stop=(k == 1),
                            )
                        fx_sb = gates.tile([128, bm], f32, tag="fx")
                        nc.scalar.activation(
                            fx_sb[:], ps_fx[:], AF.Identity,
                            bias=b_sb[:, n_lf:n_lf + 1],
                        )
                        # lf, rf: only the lh/rh chunks, then + fx on DVE
                        for g, n in ((2, n_lf), (3, n_rf)):
                            ps = psum.tile([128, bm], f32, tag="mm")
                            for k in (2, 3, 4, 5):
                                nc.tensor.matmul(
                                    ps[:], A_sb[n][:, k, :], z_sb[:, k, :],
                                    start=(k == 2), stop=(k == 5),
                                )
                            pre = tmp.tile([128, bm], f32, tag=f"pre{g}")
                            nc.vector.tensor_add(pre[:], ps[:], fx_sb[:])
                            gt = gates.tile([128, bm], f32, tag=f"g{g}")
                            nc.scalar.activation(gt[:], pre[:], AF.Sigmoid)
                            g_sb.append(gt)
                    # o: full K=768
                    n = 5 * f + 4
                    ps = psum.tile([128, bm], f32, tag="mm")
                    for k in range(KC):
                        nc.tensor.matmul(
                            ps[:], A_sb[n][:, k, :], z_sb[:, k, :],
                            start=(k == 0), stop=(k == KC - 1),
                        )
                    gt = gates.tile([128, bm], f32, tag="g4")
                    nc.scalar.activation(
                        gt[:], ps[:], AF.Sigmoid, bias=b_sb[:, n:n + 1],
                    )
                    g_sb.append(gt)

                    u_t, i_t, lf_t, rf_t, o_t = g_sb
                    t1 = tmp.tile([128, bm], f32, tag="t1")
                    nc.vector.tensor_mul(t1[:], i_t[:], u_t[:])
                    t2 = tmp.tile([128, bm], f32, tag="t2")
                    nc.vector.tensor_mul(t2[:], lf_t[:], lc_sb[:, f, :])
                    t3 = tmp.tile([128, bm], f32, tag="t3")
                    nc.vector.tensor_mul(t3[:], rf_t[:], rc_sb[:, f, :])
                    nc.vector.tensor_add(t1[:], t1[:], t2[:])
                    c_t = outp.tile([128, bm], f32, tag="c")
                    nc.vector.tensor_add(c_t[:], t1[:], t3[:])
                    eng_out = nc.sync if bm < BM else nc.gpsimd
                    eng_out.dma_start(
                        out=cT[f * 128:(f + 1) * 128, m0:m0 + bm], in_=c_t[:]
                    )
                    tc_t = tmp.tile([128, bm], f32, tag="tc")
                    nc.scalar.activation(tc_t[:], c_t[:], AF.Tanh)
                    h_t = outp.tile([128, bm], f32, tag="h")
                    nc.vector.tensor_mul(h_t[:], o_t[:], tc_t[:])
                    eng_out.dma_start(
                        out=hT[f * 128:(f + 1) * 128, m0:m0 + bm], in_=h_t[:]
                    )

    nc.compile()
    return nc


def _pack_weights(W_cx, b_cx, W_ox, b_ox, W_fx, b_fx, W_ix, b_ix,
                  U_ilh, U_irh, U_lflh, U_lfrh, U_rflh, U_rfrh,
                  U_ulh, U_urh, U_olh, U_orh):
    """A [10, 768, 128]: one [768, 128] column chunk per (half, gate),
    ordered [half0: u,i,lf,rf,o | half1: ...]; bias [128, 10] matches.
    Chunks 3 and 8 (rf) duplicate W_fx.T in rows 0:256 — the kernel never
    reads those rows except on the final block, where it substitutes the
    lf chunk's copy."""
    gates = [
        (W_cx, U_ulh, U_urh, b_cx),   # u
        (W_ix, U_ilh, U_irh, b_ix),   # i
        (W_fx, U_lflh, U_lfrh, b_fx),  # lf
        (W_fx, U_rflh, U_rfrh, b_fx),  # rf
        (W_ox, U_olh, U_orh, b_ox),   # o
    ]
    A = np.empty((10, KD, 128), dtype=np.float32)
    bias = np.empty((128, 10), dtype=np.float32)
    for g, (W, Ul, Ur, b) in enumerate(gates):
        Ag = np.concatenate([W.T, Ul.T, Ur.T], axis=0)  # [768, 256]
        for f in range(2):
            n = 5 * f + g
            A[n] = Ag[:, f * 128:(f + 1) * 128]
            bias[:, n] = b[f * 128:(f + 1) * 128]
    return np.ascontiguousarray(A), np.ascontiguousarray(bias)


def kernel(x, lc, lh, rc, rh,
           W_cx, b_cx, W_ox, b_ox, W_fx, b_fx, W_ix, b_ix,
           U_ilh, U_irh, U_lflh, U_lfrh, U_rflh, U_rfrh,
           U_ulh, U_urh, U_olh, U_orh):
    from concourse.bass_utils import run_bass_kernel_spmd

    x = np.asarray(x, dtype=np.float32)
    lc = np.asarray(lc, dtype=np.float32)
    lh = np.asarray(lh, dtype=np.float32)
    rc = np.asarray(rc, dtype=np.float32)
    rh = np.asarray(rh, dtype=np.float32)

    A, bias = _pack_weights(
        np.asarray(W_cx, np.float32), np.asarray(b_cx, np.float32),
        np.asarray(W_ox, np.float32), np.asarray(b_ox, np.float32),
        np.asarray(W_fx, np.float32), np.asarray(b_fx, np.float32),
        np.asarray(W_ix, np.float32), np.asarray(b_ix, np.float32),
        np.asarray(U_ilh, np.float32), np.asarray(U_irh, np.float32),
        np.asarray(U_lflh, np.float32), np.asarray(U_lfrh, np.float32),
        np.asarray(U_rflh, np.float32), np.asarray(U_rfrh, np.float32),
        np.asarray(U_ulh, np.float32), np.asarray(U_urh, np.float32),
        np.asarray(U_olh, np.float32), np.asarray(U_orh, np.float32),
    )

    if "nc" not in _CACHE:
        _CACHE["nc"] = _build_nc()
    nc = _CACHE["nc"]

    in_maps = []
    for c in range(CORES):
        sl = slice(c * NP_, (c + 1) * NP_)
        zTc = np.empty((KD, NP_), dtype=np.float32)
        zTc[0:D] = x[sl].T
        zTc[D:2 * D] = lh[sl].T
        zTc[2 * D:3 * D] = rh[sl].T
        in_maps.append({
            "zT": zTc,
            "lcT": np.ascontiguousarray(lc[sl].T),
            "rcT": np.ascontiguousarray(rc[sl].T),
            "A": A,
            "bias": bias,
        })

    import time as _time
    t0 = _time.time()
    res = None
    for attempt, backoff_s in ((0, 15), (1, 45), (2, None)):
        try:
            res = run_bass_kernel_spmd(nc, in_maps, core_ids=list(range(CORES)))
            break
        except Exception:
            # transient device wedge (e.g. NRT_EXEC_UNIT_UNRECOVERABLE):
            # back off and retry; re-raise on the final attempt
            if backoff_s is None:
                raise
            _time.sleep(backoff_s)
    t1 = _time.time()
    _CACHE["last_wall_s"] = t1 - t0
    _CACHE["last_exec_ns"] = res.exec_time_ns

    c_out = np.empty((N_TOTAL, D), dtype=np.float32)
    h_out = np.empty((N_TOTAL, D), dtype=np.float32)
    for ci in range(CORES):
        sl = slice(ci * NP_, (ci + 1) * NP_)
        c_out[sl] = res.results[ci]["cT"].T
        h_out[sl] = res.results[ci]["hT"].T
    return c_out, h_out



# revision 4
# speedup vs baseline: 1.5535x; 1.5535x over previous
"""BinaryTreeCell (binary tree LSTM cell) TRN2 Bass kernel.

Full-input contract: kernel(**inputs) takes the unsharded numpy inputs of
reference.setup_inputs() and returns (c, h), each [131072, 256] float32.

Strategy
--------
Data-parallel over the node dimension N=131072 across 8 NeuronCores
(16384 nodes/core); all weights replicated.

Matmuls run in fp8 (e4m3) with perf_mode=DoubleRow (2 K-rows per PE cell,
0.5 cycles per output row), with mixed-precision operand splitting to stay
inside the 2e-2 relative-error budget:

    z  = [x, lh, rh]            split as  z ~= z8 + zlo      (both e4m3)
    Ag = [W_g.T; Ul_g.T; Ur_g.T]  split as  A ~= A8 + Alo/32  (both e4m3)

  gates i, lf, rf  (sigmoid, low error sensitivity):   2 terms
      pre = z8@A8 + zlo@A8
  gates u, o       (tanh / sigmoid, high sensitivity): 3 terms
      pre = z8@A8 + zlo@A8 + (z8/32)@(32*Alo)
  The 2^-5 / 2^5 exponent shifts keep the residual weights out of the
  e4m3 denormal range (which otherwise floors the error at ~1e-2).
  Measured end-to-end rel error vs the f32 reference: ~1.5e-2.

On-chip layout is transposed (features on partitions, nodes on free dim).
Per 512-node block and 128-feature half: 36 DoubleRow matmuls accumulate
the five gates into 5 PSUM banks arranged as two 2-bank pairs (i,lf) /
(rf,o) plus one bank for u, so the four sigmoids run as two [128,1024]
activations spanning bank pairs. Gate outputs are bf16; the c/h element-
wise chain runs on VectorE in bf16 (2x DVE mode). tanh(c), h and their
stores are software-pipelined one half-block behind to keep the ACT queue
free of head-of-line waits. DMA triggers are spread across SP (z streams),
Pool/SWDGE (c,h stores), ACT (lc) and DVE (rc) so no single sequencer or
the shared HWDGE resource saturates.
"""

import numpy as np
import ml_dtypes

N_TOTAL = 131072
D = 256
CORES = 8
NP_ = N_TOTAL // CORES          # 16384 nodes per core
KD = 3 * D                      # 768 contraction
KC = KD // 128                  # 6 contraction chunks of 128
BM = 512                        # node-block (matmul free dim / PSUM bank)
NBLK = NP_ // BM                # 32 blocks per core

E4 = ml_dtypes.float8_e4m3fn
BF = ml_dtypes.bfloat16

_CACHE = {}


def _build_nc(use_bias):
    """Build + compile the per-core Bass program (same NEFF for all cores)."""
    import concourse.bass as bass
    import concourse.tile as tile
    from concourse import bacc, mybir

    f32 = mybir.dt.float32
    bf16 = mybir.dt.bfloat16
    f8 = mybir.dt.float8e4
    AF = mybir.ActivationFunctionType
    PM = mybir.MatmulPerfMode

    nc = bacc.Bacc("TRN2", target_bir_lowering=False, debug=False)

    z8T = nc.dram_tensor("z8T", [KD, NP_], f8, kind="ExternalInput").ap()
    zloT = nc.dram_tensor("zloT", [KD, NP_], f8, kind="ExternalInput").ap()
    zsT = nc.dram_tensor("zsT", [KD, NP_], f8, kind="ExternalInput").ap()
    lcT = nc.dram_tensor("lcT", [D, NP_], bf16, kind="ExternalInput").ap()
    rcT = nc.dram_tensor("rcT", [D, NP_], bf16, kind="ExternalInput").ap()
    A8 = nc.dram_tensor("A8", [KD, 10 * 128], f8, kind="ExternalInput").ap()
    Alo = nc.dram_tensor("Alo", [KD, 4 * 128], f8, kind="ExternalInput").ap()
    if use_bias:
        bias = nc.dram_tensor("bias", [128, 10], f32, kind="ExternalInput").ap()
    cT = nc.dram_tensor("cT", [D, NP_], bf16, kind="ExternalOutput").ap()
    hT = nc.dram_tensor("hT", [D, NP_], bf16, kind="ExternalOutput").ap()

    with tile.TileContext(nc) as tc:
        with (
            tc.tile_pool(name="wpool", bufs=1) as wpool,
            tc.tile_pool(name="zpool", bufs=3) as zpool,
            tc.tile_pool(name="cpool", bufs=3) as cpool,
            tc.tile_pool(name="gpool", bufs=2) as gpool,
            tc.tile_pool(name="tpool", bufs=2) as tpool,
            tc.tile_pool(name="opool", bufs=3) as opool,
            tc.tile_pool(name="psA", bufs=1, space="PSUM") as psA,
            tc.tile_pool(name="psB", bufs=1, space="PSUM") as psB,
            tc.tile_pool(name="psU", bufs=1, space="PSUM") as psU,
        ):
            # warm the activation tables (tanh + sigmoid share one set)
            warm = wpool.tile([128, 1], f32, tag="warm")
            nc.gpsimd.memset(warm[:], 0.0)
            warm_o = wpool.tile([128, 1], f32, tag="warm_o")
            nc.scalar.activation(warm_o[:], warm[:], AF.Tanh)
            nc.scalar.activation(warm_o[:], warm[:], AF.Sigmoid)

            a8_sb = wpool.tile([128, KC, 10 * 128], f8, tag="A8")
            nc.scalar.dma_start(
                out=a8_sb[:], in_=A8.rearrange("(kc p) m -> p kc m", p=128)
            )
            alo_sb = wpool.tile([128, KC, 4 * 128], f8, tag="Alo")
            nc.sync.dma_start(
                out=alo_sb[:], in_=Alo.rearrange("(kc p) m -> p kc m", p=128)
            )
            if use_bias:
                b_sb = wpool.tile([128, 10], f32, tag="b")
                nc.gpsimd.dma_start(out=b_sb[:], in_=bias[:])

            # pending (deferred) tail work from the previous (blk, f) step:
            # (c_t tile, o_slice AP, f, m0, bm)
            pend = [None]

            def flush_pending():
                if pend[0] is None:
                    return
                c_t, o_gate, f, m0, bm = pend[0]
                pend[0] = None
                tc_t = tpool.tile([128, BM], bf16, tag="tc")
                nc.scalar.activation(tc_t[:, :bm], c_t[:, :bm], AF.Tanh)
                h_t = opool.tile([128, BM], bf16, tag="h")
                nc.vector.tensor_mul(h_t[:, :bm], o_gate, tc_t[:, :bm])
                nc.gpsimd.dma_start(
                    out=hT[f * 128:(f + 1) * 128, m0:m0 + bm], in_=h_t[:, :bm]
                )

            for blk in range(NBLK):
                m0 = blk * BM
                bm = BM
                z8_sb = zpool.tile([128, KC, bm], f8, tag="z8")
                nc.sync.dma_start(
                    out=z8_sb[:],
                    in_=z8T[:, m0:m0 + bm].rearrange("(kc p) m -> p kc m", p=128),
                )
                zlo_sb = zpool.tile([128, KC, bm], f8, tag="zlo")
                nc.sync.dma_start(
                    out=zlo_sb[:],
                    in_=zloT[:, m0:m0 + bm].rearrange("(kc p) m -> p kc m", p=128),
                )
                zs_sb = zpool.tile([128, KC, bm], f8, tag="zs")
                nc.sync.dma_start(
                    out=zs_sb[:],
                    in_=zsT[:, m0:m0 + bm].rearrange("(kc p) m -> p kc m", p=128),
                )
                lc_sb = cpool.tile([128, 2, bm], bf16, tag="lc")
                nc.scalar.dma_start(
                    out=lc_sb[:],
                    in_=lcT[:, m0:m0 + bm].rearrange("(f p) m -> p f m", p=128),
                )
                rc_sb = cpool.tile([128, 2, bm], bf16, tag="rc")
                nc.sync.dma_start(
                    out=rc_sb[:],
                    in_=rcT[:, m0:m0 + bm].rearrange("(f p) m -> p f m", p=128),
                )

                for f in range(2):
                    pA_t = psA.tile([128, 2, bm], f32, tag="A")   # i, lf
                    pB_t = psB.tile([128, 2, bm], f32, tag="B")   # rf, o
                    pU_t = psU.tile([128, bm], f32, tag="U")      # u

                    def gate_mms(out_ap, g, three_term, lo_n):
                        n = 5 * f + g
                        cs = slice(n * 128, (n + 1) * 128)
                        nmm = 6 + (3 if three_term else 0)
                        k = [0]

                        def mm(a_t, z_t, cols):
                            nc.tensor.matmul(
                                out_ap, a_t[:, kp:kp + 2, cols],
                                z_t[:, kp:kp + 2, :],
                                start=(k[0] == 0), stop=(k[0] == nmm - 1),
                                perf_mode=PM.DoubleRow,
                            )
                            k[0] += 1

                        for kp in (0, 2, 4):
                            mm(a8_sb, z8_sb, cs)
                        for kp in (0, 2, 4):
                            mm(a8_sb, zlo_sb, cs)
                        if three_term:
                            cs2 = slice(lo_n * 128, (lo_n + 1) * 128)
                            for kp in (0, 2, 4):
                                mm(alo_sb, zs_sb, cs2)

                    # PSUM fill order: pair A (i, lf), pair B (rf, o), u
                    gate_mms(pA_t[:, 0, :], 1, False, 0)          # i
                    gate_mms(pA_t[:, 1, :], 2, False, 0)          # lf
                    gate_mms(pB_t[:, 0, :], 3, False, 0)          # rf
                    gate_mms(pB_t[:, 1, :], 4, True, 2 * f + 1)   # o
                    gate_mms(pU_t[:], 0, True, 2 * f + 0)         # u

                    gAB = gpool.tile([128, 2, bm], bf16, tag="gA")
                    gB = gpool.tile([128, 2, bm], bf16, tag="gB")
                    gU = gpool.tile([128, bm], bf16, tag="gU")
                    if use_bias:
                        nc.scalar.activation(
                            gAB[:, 0, :], pA_t[:, 0, :], AF.Sigmoid,
                            bias=b_sb[:, 5 * f + 1:5 * f + 2])
                        nc.scalar.activation(
                            gAB[:, 1, :], pA_t[:, 1, :], AF.Sigmoid,
                            bias=b_sb[:, 5 * f + 2:5 * f + 3])
                        nc.scalar.activation(
                            gB[:, 0, :], pB_t[:, 0, :], AF.Sigmoid,
                            bias=b_sb[:, 5 * f + 3:5 * f + 4])
                        nc.scalar.activation(
                            gB[:, 1, :], pB_t[:, 1, :], AF.Sigmoid,
                            bias=b_sb[:, 5 * f + 4:5 * f + 5])
                        nc.scalar.activation(
                            gU[:], pU_t[:], AF.Tanh,
                            bias=b_sb[:, 5 * f:5 * f + 1])
                    else:
                        nc.scalar.activation(gAB[:], pA_t[:], AF.Sigmoid)
                        nc.scalar.activation(gB[:], pB_t[:], AF.Sigmoid)
                        nc.scalar.activation(gU[:], pU_t[:], AF.Tanh)

                    # previous step's tanh(c) / h now that this step's
                    # activations are queued (keeps ACT free of HOL waits)
                    flush_pending()

                    t2 = tpool.tile([128, bm], bf16, tag="t2")
                    nc.vector.tensor_mul(t2[:], gAB[:, 1, :], lc_sb[:, f, :])
                    t3 = tpool.tile([128, bm], bf16, tag="t3")
                    nc.vector.tensor_mul(t3[:], gB[:, 0, :], rc_sb[:, f, :])
                    nc.vector.tensor_add(t2[:], t2[:], t3[:])
                    t1 = tpool.tile([128, bm], bf16, tag="t1")
                    nc.vector.tensor_mul(t1[:], gAB[:, 0, :], gU[:])
                    c_t = opool.tile([128, bm], bf16, tag="c")
                    nc.vector.tensor_add(c_t[:], t1[:], t2[:])
                    nc.gpsimd.dma_start(
                        out=cT[f * 128:(f + 1) * 128, m0:m0 + bm], in_=c_t[:]
                    )
                    pend[0] = (c_t, gB[:, 1, :], f, m0, bm)

            flush_pending()

    nc.compile()
    return nc


def _pack_weights(W_cx, W_ox, W_fx, W_ix,
                  U_ilh, U_irh, U_lflh, U_lfrh, U_rflh, U_rfrh,
                  U_ulh, U_urh, U_olh, U_orh):
    """A8 [768, 1280] e4m3: col chunk n = 5*f + g holds Ag[:, f*128:(f+1)*128]
    with Ag = [W_g.T; Ul_g.T; Ur_g.T], gates g ordered (u, i, lf, rf, o).
    Alo [768, 512] e4m3: chunks (2*f + {0:u, 1:o}) hold 32*(Ag - A8) for the
    two 3-term gates."""
    gates = [
        (W_cx, U_ulh, U_urh),    # u
        (W_ix, U_ilh, U_irh),    # i
        (W_fx, U_lflh, U_lfrh),  # lf
        (W_fx, U_rflh, U_rfrh),  # rf
        (W_ox, U_olh, U_orh),    # o
    ]
    A8 = np.empty((KD, 10 * 128), dtype=E4)
    Alo = np.empty((KD, 4 * 128), dtype=E4)
    for g, (W, Ul, Ur) in enumerate(gates):
        Ag = np.concatenate([W.T, Ul.T, Ur.T], axis=0)  # [768, 256] f32
        A8g = Ag.astype(E4)
        for f in range(2):
            A8[:, (5 * f + g) * 128:(5 * f + g + 1) * 128] = \
                A8g[:, f * 128:(f + 1) * 128]
        if g in (0, 4):
            res = (32.0 * (Ag - A8g.astype(np.float32))).astype(E4)
            n0 = 0 if g == 0 else 1
            for f in range(2):
                Alo[:, (2 * f + n0) * 128:(2 * f + n0 + 1) * 128] = \
                    res[:, f * 128:(f + 1) * 128]
    return np.ascontiguousarray(A8), np.ascontiguousarray(Alo)


def kernel(x, lc, lh, rc, rh,
           W_cx, b_cx, W_ox, b_ox, W_fx, b_fx, W_ix, b_ix,
           U_ilh, U_irh, U_lflh, U_lfrh, U_rflh, U_rfrh,
           U_ulh, U_urh, U_olh, U_orh):
    from concourse.bass_utils import run_bass_kernel_spmd

    x = np.asarray(x, dtype=np.float32)
    lc = np.asarray(lc, dtype=np.float32)
    lh = np.asarray(lh, dtype=np.float32)
    rc = np.asarray(rc, dtype=np.float32)
    rh = np.asarray(rh, dtype=np.float32)

    A8, Alo = _pack_weights(
        np.asarray(W_cx, np.float32), np.asarray(W_ox, np.float32),
        np.asarray(W_fx, np.float32), np.asarray(W_ix, np.float32),
        np.asarray(U_ilh, np.float32), np.asarray(U_irh, np.float32),
        np.asarray(U_lflh, np.float32), np.asarray(U_lfrh, np.float32),
        np.asarray(U_rflh, np.float32), np.asarray(U_rfrh, np.float32),
        np.asarray(U_ulh, np.float32), np.asarray(U_urh, np.float32),
        np.asarray(U_olh, np.float32), np.asarray(U_orh, np.float32),
    )
    biases = [np.asarray(b, np.float32) for b in (b_cx, b_ix, b_fx, b_ox)]
    use_bias = any(np.any(b) for b in biases)
    bias_pack = None
    if use_bias:
        b_cx, b_ix, b_fx, b_ox = biases
        per_gate = [b_cx, b_ix, b_fx, b_fx, b_ox]  # u, i, lf, rf, o
        bias_pack = np.empty((128, 10), dtype=np.float32)
        for g in range(5):
            for f in range(2):
                bias_pack[:, 5 * f + g] = per_gate[g][f * 128:(f + 1) * 128]

    # fp8 split of the streamed operands (e4m3 hi + e4m3 lo + 2^-5-scaled hi)
    def split(a):
        hi = a.astype(E4)
        hif = hi.astype(np.float32)
        lo = (a - hif).astype(E4)
        sc = (hif * (1.0 / 32.0)).astype(E4)
        return hi, lo, sc

    x8, xlo, xs = split(x)
    l8, llo, ls = split(lh)
    r8, rlo, rs = split(rh)
    lcb = lc.astype(BF)
    rcb = rc.astype(BF)

    key = ("nc", use_bias)
    if key not in _CACHE:
        _CACHE[key] = _build_nc(use_bias)
    nc = _CACHE[key]

    def zstack(a, b, c, sl):
        z = np.empty((KD, NP_), dtype=E4)
        z[0:D] = a[sl].T
        z[D:2 * D] = b[sl].T
        z[2 * D:3 * D] = c[sl].T
        return z

    in_maps = []
    for ci in range(CORES):
        sl = slice(ci * NP_, (ci + 1) * NP_)
        m = {
            "z8T": zstack(x8, l8, r8, sl),
            "zloT": zstack(xlo, llo, rlo, sl),
            "zsT": zstack(xs, ls, rs, sl),
            "lcT": np.ascontiguousarray(lcb[sl].T),
            "rcT": np.ascontiguousarray(rcb[sl].T),
            "A8": A8,
            "Alo": Alo,
        }
        if use_bias:
            m["bias"] = bias_pack
        in_maps.append(m)

    import time as _time
    t0 = _time.time()
    res = None
    for attempt, backoff_s in ((0, 15), (1, 45), (2, None)):
        try:
            res = run_bass_kernel_spmd(nc, in_maps, core_ids=list(range(CORES)))
            break
        except Exception:
            # transient device wedge (e.g. NRT_EXEC_UNIT_UNRECOVERABLE):
            # back off and retry; re-raise on the final attempt
            if backoff_s is None:
                raise
            _time.sleep(backoff_s)
    t1 = _time.time()
    _CACHE["last_wall_s"] = t1 - t0
    _CACHE["last_exec_ns"] = res.exec_time_ns
    _CACHE["nc"] = nc

    c_out = np.empty((N_TOTAL, D), dtype=np.float32)
    h_out = np.empty((N_TOTAL, D), dtype=np.float32)
    for ci in range(CORES):
        sl = slice(ci * NP_, (ci + 1) * NP_)
        c_out[sl] = np.asarray(res.results[ci]["cT"]).astype(np.float32).T
        h_out[sl] = np.asarray(res.results[ci]["hT"]).astype(np.float32).T
    return c_out, h_out


# revision 12
# speedup vs baseline: 1.6059x; 1.0337x over previous
"""BinaryTreeCell (binary tree LSTM cell) TRN2 Bass kernel.

Full-input contract: kernel(**inputs) takes the unsharded numpy inputs of
reference.setup_inputs() and returns (c, h), each [131072, 256] float32.

Strategy
--------
Data-parallel over the node dimension N=131072 across 8 NeuronCores
(16384 nodes/core); all weights replicated.

Matmuls run in fp8 (e4m3) with perf_mode=DoubleRow (2 K-rows per PE cell,
0.5 cycles per output row), with mixed-precision operand splitting to stay
inside the 2e-2 relative-error budget:

    z  = [x, lh, rh]            split as  z ~= z8 + zlo      (both e4m3)
    Ag = [W_g.T; Ul_g.T; Ur_g.T]  split as  A ~= A8 + Alo/32  (both e4m3)

  gate  i          (sigmoid, lowest error sensitivity): 1 term
      pre = z8@A8
  gates lf, rf     (sigmoid, low error sensitivity):    2 terms
      pre = z8@A8 + zlo@A8
  gates u, o       (tanh / sigmoid, high sensitivity):  3 terms
      pre = z8@A8 + zlo@A8 + (z8/32)@(32*Alo)
  The 2^-5 / 2^5 exponent shifts keep the residual weights out of the
  e4m3 denormal range (which otherwise floors the error at ~1e-2).
  Measured end-to-end rel error vs the f32 reference: ~1.7e-2.

On-chip layout is transposed (features on partitions, nodes on free dim).
Per 512-node block and 128-feature half, 33 DoubleRow matmuls accumulate
the five gates into 5 PSUM banks arranged as two 2-bank pairs (i,lf) /
(rf,o) plus one bank for u, so the four sigmoids run as two [128,1024]
activations spanning bank pairs. Gate outputs are bf16; the c/h elementwise chain runs on
VectorE in bf16 (2x DVE mode). c and h are stored once per block as
[128,2,bm] tiles; tanh(c) (paired across both halves), the h muls and the
h store are software-pipelined one block behind so the ACT queue never
waits head-of-line. DMA triggers are spread across SP (z streams, rc),
ACT (lc, weights) and Pool/SWDGE (c,h stores); the first block is 128
nodes so the pipeline fills fast, and the tail is split 256+128 to
shorten the exposed epilogue chain.
"""

import numpy as np
import ml_dtypes

N_TOTAL = 131072
D = 256
CORES = 8
NP_ = N_TOTAL // CORES          # 16384 nodes per core
KD = 3 * D                      # 768 contraction
KC = KD // 128                  # 6 contraction chunks of 128
NBLK_MAIN = 31

E4 = ml_dtypes.float8_e4m3fn
BF = ml_dtypes.bfloat16

_CACHE = {}


def _build_nc(use_bias):
    """Build + compile the per-core Bass program (same NEFF for all cores)."""
    import concourse.bass as bass
    import concourse.tile as tile
    from concourse import bacc, mybir

    f32 = mybir.dt.float32
    bf16 = mybir.dt.bfloat16
    f8 = mybir.dt.float8e4
    AF = mybir.ActivationFunctionType
    PM = mybir.MatmulPerfMode

    nc = bacc.Bacc("TRN2", target_bir_lowering=False, debug=False)

    z8T = nc.dram_tensor("z8T", [KD, NP_], f8, kind="ExternalInput").ap()
    zloT = nc.dram_tensor("zloT", [KD, NP_], f8, kind="ExternalInput").ap()
    zsT = nc.dram_tensor("zsT", [KD, NP_], f8, kind="ExternalInput").ap()
    lcT = nc.dram_tensor("lcT", [D, NP_], bf16, kind="ExternalInput").ap()
    rcT = nc.dram_tensor("rcT", [D, NP_], bf16, kind="ExternalInput").ap()
    A8 = nc.dram_tensor("A8", [KD, 10 * 128], f8, kind="ExternalInput").ap()
    Alo = nc.dram_tensor("Alo", [KD, 4 * 128], f8, kind="ExternalInput").ap()
    if use_bias:
        bias = nc.dram_tensor("bias", [128, 10], f32, kind="ExternalInput").ap()
    cT = nc.dram_tensor("cT", [D, NP_], bf16, kind="ExternalOutput").ap()
    hT = nc.dram_tensor("hT", [D, NP_], bf16, kind="ExternalOutput").ap()

    # node blocks: small first block for fast pipeline fill, small tail to
    # shorten the exposed epilogue
    blocks = [(0, 128)]
    off = 128
    for _ in range(NBLK_MAIN):
        blocks.append((off, 512))
        off += 512
    blocks += [(off, 256), (off + 256, 128)]
    assert off + 256 + 128 == NP_

    with tile.TileContext(nc) as tc:
        with (
            tc.tile_pool(name="wpool", bufs=1) as wpool,
            tc.tile_pool(name="zpool", bufs=3) as zpool,
            tc.tile_pool(name="cpool", bufs=3) as cpool,
            tc.tile_pool(name="gb", bufs=3) as gb,
            tc.tile_pool(name="gpool", bufs=2) as gpool,
            tc.tile_pool(name="tpool", bufs=2) as tpool,
            tc.tile_pool(name="opool", bufs=3) as opool,
            tc.tile_pool(name="psum", bufs=1, space="PSUM") as psum,
        ):
            # warm the activation tables (tanh + sigmoid share one set)
            warm = wpool.tile([128, 1], f32, tag="warm")
            nc.gpsimd.memset(warm[:], 0.0)
            warm_o = wpool.tile([128, 1], f32, tag="warm_o")
            nc.scalar.activation(warm_o[:], warm[:], AF.Tanh)
            nc.scalar.activation(warm_o[:], warm[:], AF.Sigmoid)

            a8_sb = wpool.tile([128, KC, 10 * 128], f8, tag="A8")
            alo_sb = wpool.tile([128, KC, 4 * 128], f8, tag="Alo")
            a8_src = A8.rearrange("(kc p) m -> p kc m", p=128)
            alo_src = Alo.rearrange("(kc p) m -> p kc m", p=128)

            def load_z(m0, bm):
                tiles = []
                for tag, src in (("z8", z8T), ("zlo", zloT), ("zs", zsT)):
                    t = zpool.tile([128, KC, bm], f8, tag=tag)
                    nc.sync.dma_start(
                        out=t[:],
                        in_=src[:, m0:m0 + bm].rearrange(
                            "(kc p) m -> p kc m", p=128),
                    )
                    tiles.append(t)
                lc_sb = cpool.tile([128, 2, bm], bf16, tag="lc")
                nc.scalar.dma_start(
                    out=lc_sb[:],
                    in_=lcT[:, m0:m0 + bm].rearrange("(f p) m -> p f m", p=128),
                )
                rc_sb = cpool.tile([128, 2, bm], bf16, tag="rc")
                nc.sync.dma_start(
                    out=rc_sb[:],
                    in_=rcT[:, m0:m0 + bm].rearrange("(f p) m -> p f m", p=128),
                )
                return tiles + [lc_sb, rc_sb]

            # startup order: first z block, then f0 weights, then the rest
            m0_0, bm_0 = blocks[0]
            z8_0, zlo_0, zs_0, lc_0, rc_0 = load_z(m0_0, bm_0)
            nc.scalar.dma_start(out=a8_sb[:, :, 0:640], in_=a8_src[:, :, 0:640])
            nc.scalar.dma_start(out=alo_sb[:, :, 0:256], in_=alo_src[:, :, 0:256])
            nc.scalar.dma_start(out=a8_sb[:, :, 640:1280],
                                in_=a8_src[:, :, 640:1280])
            nc.scalar.dma_start(out=alo_sb[:, :, 256:512],
                                in_=alo_src[:, :, 256:512])
            if use_bias:
                b_sb = wpool.tile([128, 10], f32, tag="b")
                nc.gpsimd.dma_start(out=b_sb[:], in_=bias[:])

            # deferred tail work from the previous block:
            # (c_pair, gIO_f0, gIO_f1, m0, bm)
            pend = [None]

            def flush_pending():
                if pend[0] is None:
                    return
                c_pair, gio0, gio1, m0, bm = pend[0]
                pend[0] = None
                tc_t = tpool.tile([128, 2, bm], bf16, tag="tc")
                nc.scalar.activation(tc_t[:], c_pair[:], AF.Tanh)
                h_t = opool.tile([128, 2, bm], bf16, tag="h")
                nc.gpsimd.tensor_mul(h_t[:, 0, :], gio0[:, 1, :], tc_t[:, 0, :])
                nc.gpsimd.tensor_mul(h_t[:, 1, :], gio1[:, 1, :], tc_t[:, 1, :])
                nc.gpsimd.dma_start(
                    out=hT[:, m0:m0 + bm].rearrange("(f p) m -> p f m", p=128),
                    in_=h_t[:],
                )

            for blk, (m0, bm) in enumerate(blocks):
                if blk == 0:
                    z8_sb, zlo_sb, zs_sb, lc_sb, rc_sb = \
                        z8_0, zlo_0, zs_0, lc_0, rc_0
                else:
                    z8_sb, zlo_sb, zs_sb, lc_sb, rc_sb = load_z(m0, bm)

                c_pair = opool.tile([128, 2, bm], bf16, tag="c")
                gbs = []
                for f in range(2):
                    # always bank-sized (512 f32) so each gate half owns a
                    # full PSUM bank: accumulation groups of different gates
                    # must not share a bank zero-region
                    pA = psum.tile([128, 2, 512], f32, tag="A")   # i, lf
                    pB = psum.tile([128, 2, 512], f32, tag="B")   # rf, o
                    pU = psum.tile([128, 512], f32, tag="U")      # u

                    zt = {0: z8_sb, 1: zlo_sb, 2: zs_sb}
                    n_i, n_lf, n_rf = 5 * f + 1, 5 * f + 2, 5 * f + 3
                    n_o, n_u = 5 * f + 4, 5 * f
                    lo_u, lo_o = 2 * f, 2 * f + 1

                    def cols(n):
                        return slice(n * 128, (n + 1) * 128)

                    entries = []

                    def gate(bank, out_ap, n, nterms, lo_n=None):
                        for term in range(nterms):
                            at = a8_sb if term < 2 else alo_sb
                            cn = cols(n) if term < 2 else cols(lo_n)
                            for kp in (0, 2, 4):
                                entries.append((bank, out_ap, at, cn,
                                                zt[term], kp, term))

                    gate("I", pA[:, 0, :bm], n_i, 1)             # z8 only
                    gate("LF", pA[:, 1, :bm], n_lf, 2)
                    gate("RF", pB[:, 0, :bm], n_rf, 2)
                    gate("O", pB[:, 1, :bm], n_o, 3, lo_o)
                    gate("U", pU[:, :bm], n_u, 3, lo_u)

                    if blk == 0:
                        # interleave by term so block 0 starts on z8 alone
                        entries.sort(key=lambda e: e[6])
                    total = {}
                    for e in entries:
                        total[e[0]] = total.get(e[0], 0) + 1
                    seen = {}
                    for bank, out_ap, at, cn, z_t, kp, term in entries:
                        k = seen.get(bank, 0)
                        seen[bank] = k + 1
                        nc.tensor.matmul(
                            out_ap, at[:, kp:kp + 2, cn], z_t[:, kp:kp + 2, :],
                            start=(k == 0), stop=(k == total[bank] - 1),
                            perf_mode=PM.DoubleRow,
                        )

                    gA = gpool.tile([128, 2, bm], bf16, tag="gA")
                    gB = gb.tile([128, 2, bm], bf16, tag="gB")
                    gU = gpool.tile([128, bm], bf16, tag="gU")
                    if use_bias:
                        nc.scalar.activation(gA[:, 0, :], pA[:, 0, :bm],
                                             AF.Sigmoid,
                                             bias=b_sb[:, n_i:n_i + 1])
                        nc.scalar.activation(gA[:, 1, :], pA[:, 1, :bm],
                                             AF.Sigmoid,
                                             bias=b_sb[:, n_lf:n_lf + 1])
                        nc.scalar.activation(gB[:, 0, :], pB[:, 0, :bm],
                                             AF.Sigmoid,
                                             bias=b_sb[:, n_lf:n_lf + 1])
                        nc.scalar.activation(gB[:, 1, :], pB[:, 1, :bm],
                                             AF.Sigmoid,
                                             bias=b_sb[:, n_o:n_o + 1])
                        nc.scalar.activation(gU[:], pU[:, :bm], AF.Tanh,
                                             bias=b_sb[:, n_u:n_u + 1])
                    else:
                        nc.scalar.activation(gA[:], pA[:, :, :bm], AF.Sigmoid)
                        nc.scalar.activation(gB[:], pB[:, :, :bm], AF.Sigmoid)
                        nc.scalar.activation(gU[:], pU[:, :bm], AF.Tanh)
                    gbs.append(gB)

                    t2 = tpool.tile([128, bm], bf16, tag="t2")
                    nc.vector.tensor_mul(t2[:], gA[:, 1, :], lc_sb[:, f, :])
                    t3 = tpool.tile([128, bm], bf16, tag="t3")
                    nc.vector.tensor_mul(t3[:], gB[:, 0, :], rc_sb[:, f, :])
                    nc.vector.tensor_add(t2[:], t2[:], t3[:])
                    t1 = tpool.tile([128, bm], bf16, tag="t1")
                    nc.vector.tensor_mul(t1[:], gA[:, 0, :], gU[:])
                    nc.vector.tensor_add(c_pair[:, f, :], t1[:], t2[:])

                    if f == 0:
                        # previous block's tanh(c), h and h-store now that this
                        # block's activations are queued (no ACT HOL waits)
                        flush_pending()

                nc.gpsimd.dma_start(
                    out=cT[:, m0:m0 + bm].rearrange("(f p) m -> p f m", p=128),
                    in_=c_pair[:],
                )
                pend[0] = (c_pair, gbs[0], gbs[1], m0, bm)

            flush_pending()

    nc.compile()
    return nc


def _pack_weights(W_cx, W_ox, W_fx, W_ix,
                  U_ilh, U_irh, U_lflh, U_lfrh, U_rflh, U_rfrh,
                  U_ulh, U_urh, U_olh, U_orh):
    """A8 [768, 1280] e4m3: col chunk n = 5*f + g holds Ag[:, f*128:(f+1)*128]
    with Ag = [W_g.T; Ul_g.T; Ur_g.T], gates g ordered (u, i, lf, rf, o).
    Alo [768, 512] e4m3: chunks (2*f + {0:u, 1:o}) hold 32*(Ag - A8) for the
    two 3-term gates."""
    gates = [
        (W_cx, U_ulh, U_urh),    # u
        (W_ix, U_ilh, U_irh),    # i
        (W_fx, U_lflh, U_lfrh),  # lf
        (W_fx, U_rflh, U_rfrh),  # rf
        (W_ox, U_olh, U_orh),    # o
    ]
    A8 = np.empty((KD, 10 * 128), dtype=E4)
    Alo = np.empty((KD, 4 * 128), dtype=E4)
    for g, (W, Ul, Ur) in enumerate(gates):
        Ag = np.concatenate([W.T, Ul.T, Ur.T], axis=0)  # [768, 256] f32
        A8g = Ag.astype(E4)
        for f in range(2):
            A8[:, (5 * f + g) * 128:(5 * f + g + 1) * 128] = \
                A8g[:, f * 128:(f + 1) * 128]
        if g in (0, 4):
            res = (32.0 * (Ag - A8g.astype(np.float32))).astype(E4)
            n0 = 0 if g == 0 else 1
            for f in range(2):
                Alo[:, (2 * f + n0) * 128:(2 * f + n0 + 1) * 128] = \
                    res[:, f * 128:(f + 1) * 128]
    return np.ascontiguousarray(A8), np.ascontiguousarray(Alo)


def kernel(x, lc, lh, rc, rh,
           W_cx, b_cx, W_ox, b_ox, W_fx, b_fx, W_ix, b_ix,
           U_ilh, U_irh, U_lflh, U_lfrh, U_rflh, U_rfrh,
           U_ulh, U_urh, U_olh, U_orh):
    from concourse.bass_utils import run_bass_kernel_spmd

    x = np.asarray(x, dtype=np.float32)
    lc = np.asarray(lc, dtype=np.float32)
    lh = np.asarray(lh, dtype=np.float32)
    rc = np.asarray(rc, dtype=np.float32)
    rh = np.asarray(rh, dtype=np.float32)

    A8, Alo = _pack_weights(
        np.asarray(W_cx, np.float32), np.asarray(W_ox, np.float32),
        np.asarray(W_fx, np.float32), np.asarray(W_ix, np.float32),
        np.asarray(U_ilh, np.float32), np.asarray(U_irh, np.float32),
        np.asarray(U_lflh, np.float32), np.asarray(U_lfrh, np.float32),
        np.asarray(U_rflh, np.float32), np.asarray(U_rfrh, np.float32),
        np.asarray(U_ulh, np.float32), np.asarray(U_urh, np.float32),
        np.asarray(U_olh, np.float32), np.asarray(U_orh, np.float32),
    )
    biases = [np.asarray(b, np.float32) for b in (b_cx, b_ix, b_fx, b_ox)]
    use_bias = any(np.any(b) for b in biases)
    bias_pack = None
    if use_bias:
        b_cx, b_ix, b_fx, b_ox = biases
        per_gate = [b_cx, b_ix, b_fx, b_fx, b_ox]  # u, i, lf, rf, o
        bias_pack = np.empty((128, 10), dtype=np.float32)
        for g in range(5):
            for f in range(2):
                bias_pack[:, 5 * f + g] = per_gate[g][f * 128:(f + 1) * 128]

    # fp8 split of the streamed operands (e4m3 hi + e4m3 lo + 2^-5-scaled hi)
    def split(a):
        hi = a.astype(E4)
        hif = hi.astype(np.float32)
        lo = (a - hif).astype(E4)
        sc = (hif * (1.0 / 32.0)).astype(E4)
        return hi, lo, sc

    x8, xlo, xs = split(x)
    l8, llo, ls = split(lh)
    r8, rlo, rs = split(rh)
    lcb = lc.astype(BF)
    rcb = rc.astype(BF)

    key = ("nc", use_bias)
    if key not in _CACHE:
        _CACHE[key] = _build_nc(use_bias)
    nc = _CACHE[key]

    def zstack(a, b, c, sl):
        z = np.empty((KD, NP_), dtype=E4)
        z[0:D] = a[sl].T
        z[D:2 * D] = b[sl].T
        z[2 * D:3 * D] = c[sl].T
        return z

    in_maps = []
    for ci in range(CORES):
        sl = slice(ci * NP_, (ci + 1) * NP_)
        m = {
            "z8T": zstack(x8, l8, r8, sl),
            "zloT": zstack(xlo, llo, rlo, sl),
            "zsT": zstack(xs, ls, rs, sl),
            "lcT": np.ascontiguousarray(lcb[sl].T),
            "rcT": np.ascontiguousarray(rcb[sl].T),
            "A8": A8,
            "Alo": Alo,
        }
        if use_bias:
            m["bias"] = bias_pack
        in_maps.append(m)

    import time as _time
    t0 = _time.time()
    res = None
    for attempt, backoff_s in ((0, 15), (1, 45), (2, None)):
        try:
            res = run_bass_kernel_spmd(nc, in_maps, core_ids=list(range(CORES)))
            break
        except Exception:
            # transient device wedge (e.g. NRT_EXEC_UNIT_UNRECOVERABLE):
            # back off and retry; re-raise on the final attempt
            if backoff_s is None:
                raise
            _time.sleep(backoff_s)
    t1 = _time.time()
    _CACHE["last_wall_s"] = t1 - t0
    _CACHE["last_exec_ns"] = res.exec_time_ns
    _CACHE["nc"] = nc

    c_out = np.empty((N_TOTAL, D), dtype=np.float32)
    h_out = np.empty((N_TOTAL, D), dtype=np.float32)
    for ci in range(CORES):
        sl = slice(ci * NP_, (ci + 1) * NP_)
        c_out[sl] = np.asarray(res.results[ci]["cT"]).astype(np.float32).T
        h_out[sl] = np.asarray(res.results[ci]["hT"]).astype(np.float32).T
    return c_out, h_out


# revision 17
# speedup vs baseline: 1.6527x; 1.0292x over previous
"""BinaryTreeCell (binary tree LSTM cell) TRN2 Bass kernel.

Full-input contract: kernel(**inputs) takes the unsharded numpy inputs of
reference.setup_inputs() and returns (c, h), each [131072, 256] float32.

Strategy
--------
Data-parallel over the node dimension N=131072 across 8 NeuronCores
(16384 nodes/core); all weights replicated.

Matmuls run in fp8 (e4m3) with perf_mode=DoubleRow (2 K-rows per PE cell,
0.5 cycles per output row), with mixed-precision operand splitting to stay
inside the 2e-2 relative-error budget:

    z  = [x, lh, rh]            split as  z ~= z8 + zlo      (both e4m3)
    Ag = [W_g.T; Ul_g.T; Ur_g.T]  split as  A ~= A8 + Alo/32  (both e4m3)

  gate  i          (sigmoid, lowest error sensitivity): 1 term
      pre = z8@A8
  gates lf, rf     (sigmoid, low error sensitivity):    2 terms
      pre = z8@A8 + zlo@A8
  gates u, o       (tanh / sigmoid, high sensitivity):  3 terms
      pre = z8@A8 + zlo@A8 + (z8/32)@(32*Alo)
  The 2^-5 / 2^5 exponent shifts keep the residual weights out of the
  e4m3 denormal range (which otherwise floors the error at ~1e-2).
  Measured end-to-end rel error vs the f32 reference: ~1.7e-2.

On-chip layout is transposed (features on partitions, nodes on free dim).
Per 512-node block and 128-feature half, 33 DoubleRow matmuls accumulate
the five gates into 5 PSUM banks arranged as two 2-bank pairs (i,lf) /
(rf,o) plus one bank for u, so the four sigmoids run as two [128,1024]
activations spanning bank pairs. Gate outputs are bf16; the c/h elementwise chain runs on
VectorE in bf16 (2x DVE mode). c and h are stored once per block as
[128,2,bm] tiles; tanh(c) (paired across both halves), the h muls and the
h store are software-pipelined one block behind so the ACT queue never
waits head-of-line. DMA triggers are spread across SP (z streams, rc),
ACT (lc, weights) and Pool/SWDGE (c,h stores); the first block is 128
nodes so the pipeline fills fast, and the tail is split 256+128 to
shorten the exposed epilogue chain.
"""

import numpy as np
import ml_dtypes

N_TOTAL = 131072
D = 256
CORES = 8
NP_ = N_TOTAL // CORES          # 16384 nodes per core
KD = 3 * D                      # 768 contraction
KC = KD // 128                  # 6 contraction chunks of 128
NBLK_MAIN = 31

E4 = ml_dtypes.float8_e4m3fn
BF = ml_dtypes.bfloat16

_CACHE = {}


def _build_nc(use_bias):
    """Build + compile the per-core Bass program (same NEFF for all cores)."""
    import concourse.bass as bass
    import concourse.tile as tile
    from concourse import bacc, mybir

    f32 = mybir.dt.float32
    bf16 = mybir.dt.bfloat16
    f8 = mybir.dt.float8e4
    AF = mybir.ActivationFunctionType
    PM = mybir.MatmulPerfMode

    nc = bacc.Bacc("TRN2", target_bir_lowering=False, debug=False)

    z8T = nc.dram_tensor("z8T", [KD, NP_], f8, kind="ExternalInput").ap()
    zloT = nc.dram_tensor("zloT", [KD, NP_], f8, kind="ExternalInput").ap()
    zsT = nc.dram_tensor("zsT", [KD, NP_], f8, kind="ExternalInput").ap()
    lcT = nc.dram_tensor("lcT", [D, NP_], bf16, kind="ExternalInput").ap()
    rcT = nc.dram_tensor("rcT", [D, NP_], bf16, kind="ExternalInput").ap()
    A8 = nc.dram_tensor("A8", [KD, 10 * 128], f8, kind="ExternalInput").ap()
    Alo = nc.dram_tensor("Alo", [KD, 4 * 128], f8, kind="ExternalInput").ap()
    if use_bias:
        bias = nc.dram_tensor("bias", [128, 10], f32, kind="ExternalInput").ap()
    cT = nc.dram_tensor("cT", [D, NP_], bf16, kind="ExternalOutput").ap()
    hT = nc.dram_tensor("hT", [D, NP_], bf16, kind="ExternalOutput").ap()

    # node blocks: small first block for fast pipeline fill, one 384 tail
    # block (with an immediate, non-deferred epilogue) to shorten the drain
    blocks = [(0, 128)]
    off = 128
    for _ in range(NBLK_MAIN):
        blocks.append((off, 512))
        off += 512
    blocks.append((off, 384))
    assert off + 384 == NP_

    with tile.TileContext(nc) as tc:
        with (
            tc.tile_pool(name="wpool", bufs=1) as wpool,
            tc.tile_pool(name="zpool", bufs=3) as zpool,
            tc.tile_pool(name="cpool", bufs=3) as cpool,
            tc.tile_pool(name="gb", bufs=3) as gb,
            tc.tile_pool(name="gpool", bufs=2) as gpool,
            tc.tile_pool(name="tpool", bufs=2) as tpool,
            tc.tile_pool(name="opool", bufs=3) as opool,
            tc.tile_pool(name="psum", bufs=1, space="PSUM") as psum,
            tc.tile_pool(name="psu2", bufs=2, space="PSUM") as psu2,
        ):
            # warm the activation tables (tanh + sigmoid share one set)
            warm = wpool.tile([128, 1], f32, tag="warm")
            nc.gpsimd.memset(warm[:], 0.0)
            warm_o = wpool.tile([128, 1], f32, tag="warm_o")
            nc.scalar.activation(warm_o[:], warm[:], AF.Tanh)
            nc.scalar.activation(warm_o[:], warm[:], AF.Sigmoid)

            a8_sb = wpool.tile([128, KC, 10 * 128], f8, tag="A8")
            alo_sb = wpool.tile([128, KC, 4 * 128], f8, tag="Alo")
            a8_src = A8.rearrange("(kc p) m -> p kc m", p=128)
            alo_src = Alo.rearrange("(kc p) m -> p kc m", p=128)

            def load_z(m0, bm):
                tiles = []
                for tag, src in (("z8", z8T), ("zlo", zloT), ("zs", zsT)):
                    t = zpool.tile([128, KC, bm], f8, tag=tag)
                    nc.sync.dma_start(
                        out=t[:],
                        in_=src[:, m0:m0 + bm].rearrange(
                            "(kc p) m -> p kc m", p=128),
                    )
                    tiles.append(t)
                lc_sb = cpool.tile([128, 2, bm], bf16, tag="lc")
                nc.scalar.dma_start(
                    out=lc_sb[:],
                    in_=lcT[:, m0:m0 + bm].rearrange("(f p) m -> p f m", p=128),
                )
                rc_sb = cpool.tile([128, 2, bm], bf16, tag="rc")
                nc.sync.dma_start(
                    out=rc_sb[:],
                    in_=rcT[:, m0:m0 + bm].rearrange("(f p) m -> p f m", p=128),
                )
                return tiles + [lc_sb, rc_sb]

            # startup order: f0 weights first (longest pole for the first
            # matmul), then the first z block, then f1 weights, then lc/rc
            nc.scalar.dma_start(out=a8_sb[:, :, 0:640], in_=a8_src[:, :, 0:640])
            m0_0, bm_0 = blocks[0]
            z8_0 = zpool.tile([128, KC, bm_0], f8, tag="z8")
            nc.sync.dma_start(
                out=z8_0[:],
                in_=z8T[:, m0_0:m0_0 + bm_0].rearrange(
                    "(kc p) m -> p kc m", p=128))
            zlo_0 = zpool.tile([128, KC, bm_0], f8, tag="zlo")
            nc.sync.dma_start(
                out=zlo_0[:],
                in_=zloT[:, m0_0:m0_0 + bm_0].rearrange(
                    "(kc p) m -> p kc m", p=128))
            zs_0 = zpool.tile([128, KC, bm_0], f8, tag="zs")
            nc.sync.dma_start(
                out=zs_0[:],
                in_=zsT[:, m0_0:m0_0 + bm_0].rearrange(
                    "(kc p) m -> p kc m", p=128))
            nc.scalar.dma_start(out=alo_sb[:, :, 0:256], in_=alo_src[:, :, 0:256])
            nc.scalar.dma_start(out=a8_sb[:, :, 640:1280],
                                in_=a8_src[:, :, 640:1280])
            nc.scalar.dma_start(out=alo_sb[:, :, 256:512],
                                in_=alo_src[:, :, 256:512])
            lc_0 = cpool.tile([128, 2, bm_0], bf16, tag="lc")
            nc.scalar.dma_start(
                out=lc_0[:],
                in_=lcT[:, m0_0:m0_0 + bm_0].rearrange(
                    "(f p) m -> p f m", p=128))
            rc_0 = cpool.tile([128, 2, bm_0], bf16, tag="rc")
            nc.sync.dma_start(
                out=rc_0[:],
                in_=rcT[:, m0_0:m0_0 + bm_0].rearrange(
                    "(f p) m -> p f m", p=128))
            if use_bias:
                b_sb = wpool.tile([128, 10], f32, tag="b")
                nc.gpsimd.dma_start(out=b_sb[:], in_=bias[:])

            # deferred tail work from the previous block:
            # (c_pair, gIO_f0, gIO_f1, m0, bm)
            pend = [None]

            def flush_pending():
                if pend[0] is None:
                    return
                c_pair, gio0, gio1, m0, bm = pend[0]
                pend[0] = None
                tc_t = tpool.tile([128, 2, bm], bf16, tag="tc")
                nc.scalar.activation(tc_t[:], c_pair[:], AF.Tanh)
                h_t = opool.tile([128, 2, bm], bf16, tag="h")
                nc.gpsimd.tensor_mul(h_t[:, 0, :], gio0[:, 1, :], tc_t[:, 0, :])
                nc.gpsimd.tensor_mul(h_t[:, 1, :], gio1[:, 1, :], tc_t[:, 1, :])
                nc.gpsimd.dma_start(
                    out=hT[:, m0:m0 + bm].rearrange("(f p) m -> p f m", p=128),
                    in_=h_t[:],
                )

            for blk, (m0, bm) in enumerate(blocks):
                if blk == 0:
                    z8_sb, zlo_sb, zs_sb, lc_sb, rc_sb = \
                        z8_0, zlo_0, zs_0, lc_0, rc_0
                else:
                    z8_sb, zlo_sb, zs_sb, lc_sb, rc_sb = load_z(m0, bm)

                c_pair = opool.tile([128, 2, bm], bf16, tag="c")
                gbs = []
                for f in range(2):
                    # always bank-sized (512 f32) so each gate half owns a
                    # full PSUM bank: accumulation groups of different gates
                    # must not share a bank zero-region
                    pA = psum.tile([128, 2, 512], f32, tag="A")   # i, lf
                    pB = psum.tile([128, 2, 512], f32, tag="B")   # rf, o
                    pU = psu2.tile([128, 512], f32, tag="U")      # u

                    zt = {0: z8_sb, 1: zlo_sb, 2: zs_sb}
                    n_i, n_lf, n_rf = 5 * f + 1, 5 * f + 2, 5 * f + 3
                    n_o, n_u = 5 * f + 4, 5 * f
                    lo_u, lo_o = 2 * f, 2 * f + 1

                    def cols(n):
                        return slice(n * 128, (n + 1) * 128)

                    entries = []

                    def gate(bank, out_ap, n, nterms, lo_n=None):
                        for term in range(nterms):
                            at = a8_sb if term < 2 else alo_sb
                            cn = cols(n) if term < 2 else cols(lo_n)
                            for kp in (0, 2, 4):
                                entries.append((bank, out_ap, at, cn,
                                                zt[term], kp, term))

                    gate("I", pA[:, 0, :bm], n_i, 1)             # z8 only
                    gate("LF", pA[:, 1, :bm], n_lf, 2)
                    gate("RF", pB[:, 0, :bm], n_rf, 2)
                    gate("O", pB[:, 1, :bm], n_o, 3, lo_o)
                    gate("U", pU[:, :bm], n_u, 3, lo_u)

                    if blk == 0:
                        # interleave by term so block 0 starts on z8 alone
                        entries.sort(key=lambda e: e[6])
                    total = {}
                    for e in entries:
                        total[e[0]] = total.get(e[0], 0) + 1
                    seen = {}
                    for bank, out_ap, at, cn, z_t, kp, term in entries:
                        k = seen.get(bank, 0)
                        seen[bank] = k + 1
                        nc.tensor.matmul(
                            out_ap, at[:, kp:kp + 2, cn], z_t[:, kp:kp + 2, :],
                            start=(k == 0), stop=(k == total[bank] - 1),
                            perf_mode=PM.DoubleRow,
                        )

                    gA = gpool.tile([128, 2, bm], bf16, tag="gA")
                    gB = gb.tile([128, 2, bm], bf16, tag="gB")
                    gU = gpool.tile([128, bm], bf16, tag="gU")
                    if use_bias:
                        nc.scalar.activation(gA[:, 0, :], pA[:, 0, :bm],
                                             AF.Sigmoid,
                                             bias=b_sb[:, n_i:n_i + 1])
                        nc.scalar.activation(gA[:, 1, :], pA[:, 1, :bm],
                                             AF.Sigmoid,
                                             bias=b_sb[:, n_lf:n_lf + 1])
                        nc.scalar.activation(gB[:, 0, :], pB[:, 0, :bm],
                                             AF.Sigmoid,
                                             bias=b_sb[:, n_lf:n_lf + 1])
                        nc.scalar.activation(gB[:, 1, :], pB[:, 1, :bm],
                                             AF.Sigmoid,
                                             bias=b_sb[:, n_o:n_o + 1])
                        nc.scalar.activation(gU[:], pU[:, :bm], AF.Tanh,
                                             bias=b_sb[:, n_u:n_u + 1])
                    else:
                        nc.scalar.activation(gA[:], pA[:, :, :bm], AF.Sigmoid)
                        nc.scalar.activation(gB[:], pB[:, :, :bm], AF.Sigmoid)
                        nc.scalar.activation(gU[:], pU[:, :bm], AF.Tanh)
                    gbs.append(gB)

                    t2 = tpool.tile([128, bm], bf16, tag="t2")
                    nc.vector.tensor_mul(t2[:], gA[:, 1, :], lc_sb[:, f, :])
                    t3 = tpool.tile([128, bm], bf16, tag="t3")
                    nc.vector.tensor_mul(t3[:], gB[:, 0, :], rc_sb[:, f, :])
                    nc.vector.tensor_add(t2[:], t2[:], t3[:])
                    t1 = tpool.tile([128, bm], bf16, tag="t1")
                    nc.vector.tensor_mul(t1[:], gA[:, 0, :], gU[:])
                    nc.vector.tensor_add(c_pair[:, f, :], t1[:], t2[:])

                    if blk == len(blocks) - 1:
                        # tail block: immediate per-half epilogue on ACT+DVE
                        # (short exposed chain, no Pool in the critical path)
                        nc.gpsimd.dma_start(
                            out=cT[f * 128:(f + 1) * 128, m0:m0 + bm],
                            in_=c_pair[:, f, :],
                        )
                        tcf = tpool.tile([128, bm], bf16, tag="tcf")
                        nc.scalar.activation(tcf[:], c_pair[:, f, :], AF.Tanh)
                        hf = opool.tile([128, bm], bf16, tag="hf")
                        nc.vector.tensor_mul(hf[:], gB[:, 1, :], tcf[:])
                        nc.sync.dma_start(
                            out=hT[f * 128:(f + 1) * 128, m0:m0 + bm],
                            in_=hf[:],
                        )
                    if f == 0:
                        # previous block's tanh(c), h and h-store now that this
                        # block's activations are queued (no ACT HOL waits)
                        flush_pending()

                if blk < len(blocks) - 1:
                    nc.gpsimd.dma_start(
                        out=cT[:, m0:m0 + bm].rearrange(
                            "(f p) m -> p f m", p=128),
                        in_=c_pair[:],
                    )
                    pend[0] = (c_pair, gbs[0], gbs[1], m0, bm)

            flush_pending()

    nc.compile()
    return nc


def _pack_weights(W_cx, W_ox, W_fx, W_ix,
                  U_ilh, U_irh, U_lflh, U_lfrh, U_rflh, U_rfrh,
                  U_ulh, U_urh, U_olh, U_orh):
    """A8 [768, 1280] e4m3: col chunk n = 5*f + g holds Ag[:, f*128:(f+1)*128]
    with Ag = [W_g.T; Ul_g.T; Ur_g.T], gates g ordered (u, i, lf, rf, o).
    Alo [768, 512] e4m3: chunks (2*f + {0:u, 1:o}) hold 32*(Ag - A8) for the
    two 3-term gates."""
    gates = [
        (W_cx, U_ulh, U_urh),    # u
        (W_ix, U_ilh, U_irh),    # i
        (W_fx, U_lflh, U_lfrh),  # lf
        (W_fx, U_rflh, U_rfrh),  # rf
        (W_ox, U_olh, U_orh),    # o
    ]
    A8 = np.empty((KD, 10 * 128), dtype=E4)
    Alo = np.empty((KD, 4 * 128), dtype=E4)
    for g, (W, Ul, Ur) in enumerate(gates):
        Ag = np.concatenate([W.T, Ul.T, Ur.T], axis=0)  # [768, 256] f32
        A8g = Ag.astype(E4)
        for f in range(2):
            A8[:, (5 * f + g) * 128:(5 * f + g + 1) * 128] = \
                A8g[:, f * 128:(f + 1) * 128]
        if g in (0, 4):
            res = (32.0 * (Ag - A8g.astype(np.float32))).astype(E4)
            n0 = 0 if g == 0 else 1
            for f in range(2):
                Alo[:, (2 * f + n0) * 128:(2 * f + n0 + 1) * 128] = \
                    res[:, f * 128:(f + 1) * 128]
    return np.ascontiguousarray(A8), np.ascontiguousarray(Alo)


def kernel(x, lc, lh, rc, rh,
           W_cx, b_cx, W_ox, b_ox, W_fx, b_fx, W_ix, b_ix,
           U_ilh, U_irh, U_lflh, U_lfrh, U_rflh, U_rfrh,
           U_ulh, U_urh, U_olh, U_orh):
    from concourse.bass_utils import run_bass_kernel_spmd

    x = np.asarray(x, dtype=np.float32)
    lc = np.asarray(lc, dtype=np.float32)
    lh = np.asarray(lh, dtype=np.float32)
    rc = np.asarray(rc, dtype=np.float32)
    rh = np.asarray(rh, dtype=np.float32)

    A8, Alo = _pack_weights(
        np.asarray(W_cx, np.float32), np.asarray(W_ox, np.float32),
        np.asarray(W_fx, np.float32), np.asarray(W_ix, np.float32),
        np.asarray(U_ilh, np.float32), np.asarray(U_irh, np.float32),
        np.asarray(U_lflh, np.float32), np.asarray(U_lfrh, np.float32),
        np.asarray(U_rflh, np.float32), np.asarray(U_rfrh, np.float32),
        np.asarray(U_ulh, np.float32), np.asarray(U_urh, np.float32),
        np.asarray(U_olh, np.float32), np.asarray(U_orh, np.float32),
    )
    biases = [np.asarray(b, np.float32) for b in (b_cx, b_ix, b_fx, b_ox)]
    use_bias = any(np.any(b) for b in biases)
    bias_pack = None
    if use_bias:
        b_cx, b_ix, b_fx, b_ox = biases
        per_gate = [b_cx, b_ix, b_fx, b_fx, b_ox]  # u, i, lf, rf, o
        bias_pack = np.empty((128, 10), dtype=np.float32)
        for g in range(5):
            for f in range(2):
                bias_pack[:, 5 * f + g] = per_gate[g][f * 128:(f + 1) * 128]

    # fp8 split of the streamed operands (e4m3 hi + e4m3 lo + 2^-5-scaled hi)
    def split(a):
        hi = a.astype(E4)
        hif = hi.astype(np.float32)
        lo = (a - hif).astype(E4)
        sc = (hif * (1.0 / 32.0)).astype(E4)
        return hi, lo, sc

    x8, xlo, xs = split(x)
    l8, llo, ls = split(lh)
    r8, rlo, rs = split(rh)
    lcb = lc.astype(BF)
    rcb = rc.astype(BF)

    key = ("nc", use_bias)
    if key not in _CACHE:
        _CACHE[key] = _build_nc(use_bias)
    nc = _CACHE[key]

    def zstack(a, b, c, sl):
        z = np.empty((KD, NP_), dtype=E4)
        z[0:D] = a[sl].T
        z[D:2 * D] = b[sl].T
        z[2 * D:3 * D] = c[sl].T
        return z

    in_maps = []
    for ci in range(CORES):
        sl = slice(ci * NP_, (ci + 1) * NP_)
        m = {
            "z8T": zstack(x8, l8, r8, sl),
            "zloT": zstack(xlo, llo, rlo, sl),
            "zsT": zstack(xs, ls, rs, sl),
            "lcT": np.ascontiguousarray(lcb[sl].T),
            "rcT": np.ascontiguousarray(rcb[sl].T),
            "A8": A8,
            "Alo": Alo,
        }
        if use_bias:
            m["bias"] = bias_pack
        in_maps.append(m)

    import time as _time
    t0 = _time.time()
    res = None
    for attempt, backoff_s in ((0, 15), (1, 45), (2, None)):
        try:
            res = run_bass_kernel_spmd(nc, in_maps, core_ids=list(range(CORES)))
            break
        except Exception:
            # transient device wedge (e.g. NRT_EXEC_UNIT_UNRECOVERABLE):
            # back off and retry; re-raise on the final attempt
            if backoff_s is None:
                raise
            _time.sleep(backoff_s)
    t1 = _time.time()
    _CACHE["last_wall_s"] = t1 - t0
    _CACHE["last_exec_ns"] = res.exec_time_ns
    _CACHE["nc"] = nc

    c_out = np.empty((N_TOTAL, D), dtype=np.float32)
    h_out = np.empty((N_TOTAL, D), dtype=np.float32)
    for ci in range(CORES):
        sl = slice(ci * NP_, (ci + 1) * NP_)
        c_out[sl] = np.asarray(res.results[ci]["cT"]).astype(np.float32).T
        h_out[sl] = np.asarray(res.results[ci]["hT"]).astype(np.float32).T
    return c_out, h_out
